# revision 25
# baseline (speedup 1.0000x reference)
"""LSTM-pool kernel for Trainium2, 8-core data-parallel SPMD.

Math (per batch row b):
  x_t = [seq[b,t], seq_e[b,t], seq_t[b,t]]              (A = 384)
  z_t = x_t @ Wi + h_{t-1} @ Wh + bh                    (4F = 512, gates i,f,g,o)
  c_t = sig(f)*c_{t-1} + sig(i)*tanh(g);  h_t = sig(o)*tanh(c_t)
  out = relu([h_T, src] @ W1 + b1) @ W2 + b2

Implementation notes (steady state is ACT-bound at ~3.69us/step, ACT
99.9% busy = the structural floor for this dataflow):
  * Host pre-transposes the 3 big [B,T,F] tensors to feature-major fp8e4m3
    (x/4, Wi*4 so PSUM holds the exact product) - no on-device casts or
    transposes, 4x less HBM traffic.
  * All z matmuls are fp8 DoubleRow; the pad k-half of each DR pass is a
    stride-0 AP that re-reads real data against an all-zero weight half,
    so no zero-pad planes or memsets exist on device.
  * PSUM: two [128, 4, 256] pair tiles (2 banks each), double buffered =
    all 8 banks.  Exactly one start=True per bank per step.
  * The g-quad columns of Wi/Wh are pre-scaled by 2 on the host so a
    single Sigmoid covers all four gates; tanh(g) = 2*sig(2g)-1 is
    reconstructed on the DVE (one tensor_scalar op).  ACT runs exactly
    4 sigmoids (PSUM-src, FD=512) + 4 tanhs (SBUF-src, FD=128) per step.
  * Batch 512/core is pipelined as K=4 chunks of 128 processed in slot
    order [0,2,1,3], so concurrent rec matmuls never write the PSUM bank
    the sigmoid is reading.  Each chunk's tanh tail runs two slots late
    and is emitted before the slot's sigmoid: the ACT in-order queue then
    always holds ready work while a sigmoid waits on its rec matmuls.
  * The cell adds are deferred one slot (no DVE RAW pipe stall) and the
    h-mul runs last-in-slot on the DVE - NOT gpsimd, whose shared SBUF
    port would steal ~200ns from whatever DVE op overlaps it.
  * Gate/cell arithmetic is bf16 on the DVE; h is written in fp8 for the
    recurrent matmul, except the last step which writes bf16 straight
    into the merge-layer input (better precision, no copies).
  * Prologue: DMA issue order is latency-critical (each dma_start costs
    ~650ns serialized on the Sync queue): wi first, then the first 4 time
    steps of x, then the rest; merge-layer constants last.  ~10 junk
    matmuls on scratch tiles warm the PE HAM clock gate to 2.4GHz while
    the DMAs land.
"""

import sys

sys.path.insert(0, "/opt/trn_rl_repo")

import numpy as np

import concourse.bass as bass
import concourse.mybir as mybir
import concourse.tile as tile
from concourse import bacc
from concourse.bass_utils import run_bass_kernel_spmd

dt = mybir.dt
AF = mybir.ActivationFunctionType
ALU = mybir.AluOpType
F8 = dt.np(dt.float8e4)
BF16 = dt.np(dt.bfloat16)

NCORES = 8
BFULL = 4096
B = BFULL // NCORES  # 512 batch rows per core
T = 128
F = 128
K = 4  # batch chunks per core
NH = B // K  # 128 rows per chunk
TC = 16  # time steps per DMA chunk
XSCALE = 4.0  # x shipped as x/XSCALE in fp8, Wi as Wi*XSCALE

# PSUM quad-bank order [g, i, f, o]; quad q -> Wi/Wh column block index
QUAD_COLS = [2, 0, 1, 3]


def build_nc(zero_bias: bool, t_steps: int = T):
    nc = bacc.Bacc("TRN2", target_bir_lowering=False, debug=False, num_devices=NCORES)

    xT = nc.dram_tensor("xT", [3, 128, T, B], dt.float8e4, kind="ExternalInput")
    wiP = nc.dram_tensor("wiP", [4, 2, 128, 2, 128], dt.float8e4, kind="ExternalInput")
    whP = nc.dram_tensor("whP", [4, 128, 2, 128], dt.float8e4, kind="ExternalInput")
    bh4 = nc.dram_tensor("bh4", [128, 4], dt.float32, kind="ExternalInput")
    srcT = nc.dram_tensor("srcT", [128, B], dt.bfloat16, kind="ExternalInput")
    w1b = nc.dram_tensor("w1b", [2, 128, 128], dt.bfloat16, kind="ExternalInput")
    w2b = nc.dram_tensor("w2b", [128, 128], dt.bfloat16, kind="ExternalInput")
    b1 = nc.dram_tensor("b1", [128], dt.float32, kind="ExternalInput")
    b2 = nc.dram_tensor("b2", [128], dt.float32, kind="ExternalInput")
    outT = nc.dram_tensor("outT", [F, B], dt.float32, kind="ExternalOutput")

    nchunk = (t_steps + TC - 1) // TC
    DR = mybir.MatmulPerfMode.DoubleRow

    with tile.TileContext(nc) as tc:
        with (
            tc.tile_pool(name="const", bufs=1) as constp,
            tc.tile_pool(name="gates", bufs=3) as gatep,
        ):
            # ---------------- weights / constants ----------------
            # DMA issue order matters: each dma_start costs ~650ns serialized
            # on the Sync queue, so the tensors gating the first input
            # projection (wi, first x steps) go first; merge-layer constants
            # are only needed ~500us later and go last.
            # junkw/junkx feed PE warm-up matmuls (values irrelevant; Tile
            # requires a write before any read)
            junkw = constp.tile([128, 128], dt.float8e4, name="junkw")
            junkx = constp.tile([128, 512], dt.float8e4, name="junkx")
            nc.gpsimd.memset(junkw[:], 0.0)
            nc.gpsimd.memset(junkx[:], 0.0)

            wi = constp.tile([128, 4, 2, 2, 128], dt.float8e4)
            nc.sync.dma_start(wi[:], wiP[:].rearrange("q pr k two m -> k q pr two m"))

            # ---------------- x staging (double buffer) ----------------
            # No DoubleRow zero-pad plane: the pad half of each DR pass uses
            # a stride-0 AP that re-reads the real data, which is multiplied
            # by the zero weight half (host packs zeros there), so its value
            # never matters and no memset is needed.
            xts = []
            for i in range(2):
                xt = constp.tile([128, 3, TC, B], dt.float8e4, name=f"xt{i}")
                xts.append(xt)

            xsrc = xT[:].rearrange("kc p t b -> p kc t b")
            nc.sync.dma_start(xts[0][:, :, 0:4, :], xsrc[:, :, 0:4, :])
            nc.sync.dma_start(xts[0][:, :, 4:10, :], xsrc[:, :, 4:10, :])

            wh = constp.tile([128, 4, 2, 128], dt.float8e4)
            nc.sync.dma_start(wh[:], whP[:].rearrange("q k two m -> k q two m"))
            nc.sync.dma_start(xts[0][:, :, 10:TC, :], xsrc[:, :, 10:TC, :])

            srcb = constp.tile([128, B], dt.bfloat16)
            nc.sync.dma_start(srcb[:], srcT[:])
            w1 = constp.tile([128, 2, 128], dt.bfloat16)
            nc.sync.dma_start(w1[:], w1b[:].rearrange("two k m -> k two m"))
            w2 = constp.tile([128, 128], dt.bfloat16)
            nc.sync.dma_start(w2[:], w2b[:])
            b1t = constp.tile([128, 1], dt.float32)
            nc.sync.dma_start(b1t[:], b1[:].rearrange("(f one) -> f one", one=1))
            b2t = constp.tile([128, 1], dt.float32)
            nc.sync.dma_start(b2t[:], b2[:].rearrange("(f one) -> f one", one=1))
            bias_g = None
            if not zero_bias:
                bias_g = constp.tile([128, 4], dt.float32)
                nc.sync.dma_start(bias_g[:], bh4[:])

            def dma_chunk(ch):
                t0 = ch * TC
                nc.sync.dma_start(
                    xts[ch % 2][:], xsrc[:, :, t0 : t0 + TC, :]
                )

            def dr_pad(ap):
                """[p, n] AP -> [p, 2, n] with a stride-0 middle dim (the
                DoubleRow second k-tile re-reads the same data; the matching
                stationary half is all zeros)."""
                out = bass.AP(ap.tensor, ap.offset, [ap.ap[0], [0, 2], *ap.ap[1:]])
                return out

            # ---------------- persistent state ----------------
            # hs needs no init: hmul(c, t=0) always writes it before the
            # first rec (t=1) reads it.
            cs, hs = [], []
            for c in range(K):
                c_t = constp.tile([128, NH], dt.bfloat16, name=f"c_{c}")
                nc.gpsimd.memset(c_t[:], 0.0)
                cs.append(c_t)
                hs.append(constp.tile([128, NH], dt.float8e4, name=f"h_{c}"))
            # merge-layer input: the last step's h in bf16, written directly
            # by the final hmul of each chunk
            hbf = constp.tile([128, K, NH], dt.bfloat16)

            zp_ctx = tc.tile_pool(name="zp", bufs=2, space="PSUM")
            zp = zp_ctx.__enter__()

            def emit_ip_pair(zt, t, pr, with_stop):
                """input projection for step t, chunk-pair pr (N=256).

                The pair tile [128, 4, 256] spans two banks (quads g,i and
                f,o); one start per bank (q0/q2), stop on the pair's last
                recurrent matmul per bank (or ip q1/q3 at t=0).
                """
                buf = xts[(t // TC) % 2]
                ts_ = t % TC
                bs = slice(pr * 2 * NH, (pr + 1) * 2 * NH)
                for q in range(4):
                    nc.tensor.matmul(
                        zt[:, q, :],
                        wi[:, q, 0, :, :],
                        buf[:, 0:2, ts_, bs],
                        start=(q in (0, 2)),
                        stop=False,
                        perf_mode=DR,
                        skip_group_check=True,
                    )
                    nc.tensor.matmul(
                        zt[:, q, :],
                        wi[:, q, 1, :, :],
                        dr_pad(buf[:, 2, ts_, bs]),
                        start=False,
                        stop=(with_stop and q in (1, 3)),
                        perf_mode=DR,
                        skip_group_check=True,
                    )

            def emit_rec(zt, c):
                half = slice((c % 2) * NH, (c % 2 + 1) * NH)
                for q in range(4):
                    nc.tensor.matmul(
                        zt[:, q, half],
                        wh[:, q, :, :],
                        dr_pad(hs[c][:]),
                        start=False,
                        stop=(c % 2 == 1 and q in (1, 3)),
                        perf_mode=DR,
                        skip_group_check=True,
                    )

            def emit_gates(zt, t, c):
                """single sigmoid over all 4 quads (g pre-scaled by 2)."""
                half = slice((c % 2) * NH, (c % 2 + 1) * NH)
                sg = gatep.tile(
                    [128, 4, NH], dt.bfloat16, tag=f"sg{c}", name=f"sg{c}_{t}"
                )
                if zero_bias:
                    nc.scalar.activation(sg[:], zt[:, :, half], AF.Sigmoid)
                else:
                    for q in range(4):
                        nc.scalar.activation(
                            sg[:, q, :],
                            zt[:, q, half],
                            AF.Sigmoid,
                            bias=bias_g[:, q : q + 1],
                        )
                return sg

            def emit_cell(t, c, sg):
                # tanh(g) = 2*sig(2g) - 1 (the 2x is pre-folded into the
                # g-quad weights).  Order ts, m1, m2 so no op reads a tile
                # written by the immediately preceding DVE op (the RAW pipe
                # stall costs ~90ns); the final add is deferred one slot for
                # the same reason (see emit_cell_add).
                tg = gatep.tile([128, NH], dt.bfloat16, tag=f"tg{c}", name=f"tg{c}_{t}")
                nc.vector.tensor_scalar(tg[:], sg[:, 0, :], 2.0, -1.0, ALU.mult, ALU.add)
                m1 = gatep.tile([128, NH], dt.bfloat16, tag=f"m1{c}", name=f"m1{c}_{t}")
                nc.vector.tensor_mul(m1[:], sg[:, 2, :], cs[c][:])
                m2 = gatep.tile([128, NH], dt.bfloat16, tag=f"m2{c}", name=f"m2{c}_{t}")
                nc.vector.tensor_mul(m2[:], sg[:, 1, :], tg[:])
                return (m1, m2)

            def emit_cell_add(c, m1, m2):
                nc.vector.tensor_add(cs[c][:], m1[:], m2[:])

            def emit_tail_tanh(t, c, sg):
                tc2 = gatep.tile(
                    [128, NH], dt.bfloat16, tag=f"tc{c}", name=f"tc{c}_{t}"
                )
                nc.scalar.activation(tc2[:], cs[c][:], AF.Tanh)
                return tc2

            def emit_tail_hmul(t, c, sg, tc2):
                # h-mul on the DVE, placed after the slot's cell ops: running
                # it on gpsimd contends with concurrent DVE ops for the
                # shared POOL SBUF port (+200ns on whichever DVE op overlaps).
                # The final step's h goes straight to the merge input in bf16.
                dst = hbf[:, c, :] if t == t_steps - 1 else hs[c][:]
                nc.vector.tensor_mul(dst, sg[:, 3, :], tc2[:])

            # PE warm-up: ~12 junk matmuls on uninitialized scratch get the
            # HAM clock gate to 2.4GHz (~3.4us of PE activity) while the
            # first DMAs land; the first real projection then runs warm.
            # Each is its own closed accumulation group into a region the
            # real ip overwrites with start=True.
            z_cur = []
            for pr in range(K // 2):
                zt = zp.tile([128, 4, 2 * NH], dt.float32, tag=f"zp{pr}", name=f"zp{pr}_p")
                z_cur.append(zt)
            for _ in range(10):
                nc.tensor.matmul(
                    z_cur[0][:, 0:2, :],
                    junkw[:],
                    junkx[:],
                    start=True,
                    stop=True,
                    skip_group_check=True,
                )

            # prologue: projections for t=0
            for pr in range(K // 2):
                emit_ip_pair(z_cur[pr], 0, pr, with_stop=True)

            # Chunk slot order [0,2,1,3]: the rec matmuls running concurrently
            # with sig(c) then always target the OTHER pair's PSUM banks, so
            # the ACT PSUM read never contends with PE bank writes.
            CH = [0, 2, 1, 3]
            # Tails run two slots late and are emitted BEFORE the slot's sig,
            # so the ACT queue always holds ready work (the tanh) while the
            # sig waits on its rec matmuls, and each chunk's rec lands exactly
            # two slots after its h-mul.
            tails = []  # (t, c, sg) awaiting tail emission
            adds = []  # (c, m1, m2) awaiting the deferred cs add
            for t in range(t_steps):
                ch = t // TC
                if t % TC == 0 and ch + 1 < nchunk:
                    dma_chunk(ch + 1)
                for ci in range(K):
                    c = CH[ci]
                    if adds:
                        emit_cell_add(*adds.pop(0))
                    hm = None
                    if len(tails) >= 2:
                        hm = tails.pop(0)
                        tc2 = emit_tail_tanh(*hm)
                    zt = z_cur[c // 2]
                    if t > 0:
                        emit_rec(zt, c)
                    # pair ip for t+1 is emitted at the pair's second chunk
                    # slot (both chunks' step t-1 gate reads are then long
                    # retired, so the fresh tile never stalls the PE).
                    if t + 1 < t_steps and c % 2 == 1:
                        pr = c // 2
                        zn = zp.tile(
                            [128, 4, 2 * NH],
                            dt.float32,
                            tag=f"zp{pr}",
                            name=f"zp{pr}_{t + 1}",
                        )
                        emit_ip_pair(zn, t + 1, pr, with_stop=False)
                        z_cur[pr] = zn
                    sg = emit_gates(zt, t, c)
                    m1, m2 = emit_cell(t, c, sg)
                    if hm is not None:
                        emit_tail_hmul(*hm, tc2)
                    adds.append((c, m1, m2))
                    tails.append((t, c, sg))
            if adds:
                emit_cell_add(*adds.pop(0))
            for p in tails:
                tc2 = emit_tail_tanh(*p)
                emit_tail_hmul(*p, tc2)

            zp_ctx.__exit__(None, None, None)

            # ---------------- merge layer ----------------
            with tc.tile_pool(name="mp", bufs=1, space="PSUM") as mp:
                ps_hid = mp.tile([128, B], dt.float32)
                for c in range(K):
                    bs = slice(c * NH, (c + 1) * NH)
                    nc.tensor.matmul(
                        ps_hid[:, bs], w1[:, 0, :], hbf[:, c, :], start=True, stop=False
                    )
                    nc.tensor.matmul(
                        ps_hid[:, bs], w1[:, 1, :], srcb[:, bs], start=False, stop=True
                    )
                hid_bf = constp.tile([128, B], dt.bfloat16)
                nc.scalar.activation(hid_bf[:], ps_hid[:], AF.Relu, bias=b1t[:])

                ps_out = mp.tile([128, B], dt.float32)
                nc.tensor.matmul(ps_out[:], w2[:], hid_bf[:], start=True, stop=True)
                out_sb = constp.tile([128, B], dt.float32)
                nc.scalar.activation(out_sb[:], ps_out[:], AF.Identity, bias=b2t[:])
                nc.sync.dma_start(outT[:], out_sb[:])

    nc.compile()
    return nc


_NC_CACHE: dict = {}


def _get_nc(zero_bias: bool):
    if zero_bias not in _NC_CACHE:
        _NC_CACHE[zero_bias] = build_nc(zero_bias)
    return _NC_CACHE[zero_bias]


def make_in_maps(**inputs):
    """Host-side reshaping: slice per core, pre-transpose, pre-quantize."""
    f32 = lambda x: np.asarray(x, dtype=np.float32)
    Wi = f32(inputs["Wi"])  # [384, 512]
    Wh = f32(inputs["Wh"])  # [128, 512]
    bh = f32(inputs["bh"])  # [512]
    W1 = f32(inputs["W1"])  # [256, 128]
    W2 = f32(inputs["W2"])  # [128, 128]
    b1 = f32(inputs["b1"])
    b2 = f32(inputs["b2"])

    # Wi packed for DoubleRow: [q, pair, k, two, m], scaled by XSCALE.
    # Wh packed for DoubleRow with a zero second k-tile: [q, k, two, m].
    # The g quad (and its bias) is additionally scaled by 2 so that
    # tanh(g) = 2*sigmoid(2g) - 1 comes out of the shared sigmoid.
    wiP = np.zeros((4, 2, 128, 2, 128), np.float32)
    whP = np.zeros((4, 128, 2, 128), np.float32)
    bh4 = np.zeros((128, 4), np.float32)
    for q, blk in enumerate(QUAD_COLS):
        gs = 2.0 if q == 0 else 1.0
        colsl = slice(blk * 128, (blk + 1) * 128)
        for kc in range(3):
            wiP[q, kc // 2, :, kc % 2, :] = (
                gs * XSCALE * Wi[kc * 128 : (kc + 1) * 128, colsl]
            )
        whP[q, :, 0, :] = gs * Wh[:, colsl]
        bh4[:, q] = gs * bh[colsl]
    wiP = wiP.astype(F8)
    whP = whP.astype(F8)
    w1b = np.stack([W1[0:128, :], W1[128:256, :]]).astype(BF16)
    w2b = W2.astype(BF16)

    shared = {
        "wiP": wiP,
        "whP": whP,
        "bh4": np.ascontiguousarray(bh4),
        "w1b": w1b,
        "w2b": w2b,
        "b1": b1,
        "b2": b2,
    }

    # big tensors: cast full arrays to fp8 once, then per-core transpose
    planes = []
    for nm in ("seq", "seq_e", "seq_t"):
        a = np.asarray(inputs[nm])
        planes.append((a * (1.0 / XSCALE)).astype(F8))  # [4096, T, F]
    src = f32(inputs["src"])

    in_maps = []
    for c in range(NCORES):
        sl = slice(c * B, (c + 1) * B)
        m = dict(shared)
        xT = np.empty((3, 128, T, B), F8)
        for kc in range(3):
            xT[kc] = planes[kc][sl].transpose(2, 1, 0)
        m["xT"] = xT
        m["srcT"] = np.ascontiguousarray(src[sl].T).astype(BF16)
        in_maps.append(m)
    return in_maps


def kernel(**inputs) -> np.ndarray:
    zero_bias = not np.any(np.asarray(inputs["bh"]))
    nc = _get_nc(zero_bias)
    in_maps = make_in_maps(**inputs)
    res = run_bass_kernel_spmd(nc, in_maps, core_ids=list(range(NCORES)))
    out = np.empty((BFULL, F), np.float32)
    for c in range(NCORES):
        out[c * B : (c + 1) * B] = res.results[c]["outT"].T
    return out



# revision 27
# speedup vs baseline: 1.0031x; 1.0031x over previous
"""LSTM-pool kernel for Trainium2, 8-core data-parallel SPMD.

Math (per batch row b):
  x_t = [seq[b,t], seq_e[b,t], seq_t[b,t]]              (A = 384)
  z_t = x_t @ Wi + h_{t-1} @ Wh + bh                    (4F = 512, gates i,f,g,o)
  c_t = sig(f)*c_{t-1} + sig(i)*tanh(g);  h_t = sig(o)*tanh(c_t)
  out = relu([h_T, src] @ W1 + b1) @ W2 + b2

Implementation notes (steady state is ACT-bound at ~3.69us/step, ACT
99.9% busy = the structural floor for this dataflow):
  * Host pre-transposes the 3 big [B,T,F] tensors to feature-major fp8e4m3
    (x/4, Wi*4 so PSUM holds the exact product) - no on-device casts or
    transposes, 4x less HBM traffic.
  * All z matmuls are fp8 DoubleRow; the pad k-half of each DR pass is a
    stride-0 AP that re-reads real data against an all-zero weight half,
    so no zero-pad planes or memsets exist on device.
  * PSUM: two [128, 4, 256] pair tiles (2 banks each), double buffered =
    all 8 banks.  Exactly one start=True per bank per step.
  * The g-quad columns of Wi/Wh are pre-scaled by 2 on the host so a
    single Sigmoid covers all four gates; tanh(g) = 2*sig(2g)-1 is
    reconstructed on the DVE (one tensor_scalar op).  ACT runs exactly
    4 sigmoids (PSUM-src, FD=512) + 4 tanhs (SBUF-src, FD=128) per step.
  * Batch 512/core is pipelined as K=4 chunks of 128 processed in slot
    order [0,2,1,3], so concurrent rec matmuls never write the PSUM bank
    the sigmoid is reading.  Each chunk's tanh tail runs two slots late
    and is emitted before the slot's sigmoid: the ACT in-order queue then
    always holds ready work while a sigmoid waits on its rec matmuls.
  * The cell adds are deferred one slot (no DVE RAW pipe stall) and the
    h-mul runs last-in-slot on the DVE - NOT gpsimd, whose shared SBUF
    port would steal ~200ns from whatever DVE op overlaps it.
  * Gate/cell arithmetic is bf16 on the DVE; h is written in fp8 for the
    recurrent matmul, except the last step which writes bf16 straight
    into the merge-layer input (better precision, no copies).
  * Prologue: DMA issue order is latency-critical (each dma_start costs
    ~650ns serialized on the Sync queue): wi first, then the first 4 time
    steps of x, then the rest; merge-layer constants last.  ~10 junk
    matmuls on scratch tiles warm the PE HAM clock gate to 2.4GHz while
    the DMAs land.
"""

import sys

sys.path.insert(0, "/opt/trn_rl_repo")

import numpy as np

import concourse.bass as bass
import concourse.mybir as mybir
import concourse.tile as tile
from concourse import bacc
from concourse.bass_utils import run_bass_kernel_spmd

dt = mybir.dt
AF = mybir.ActivationFunctionType
ALU = mybir.AluOpType
F8 = dt.np(dt.float8e4)
BF16 = dt.np(dt.bfloat16)

NCORES = 8
BFULL = 4096
B = BFULL // NCORES  # 512 batch rows per core
T = 128
F = 128
K = 4  # batch chunks per core
NH = B // K  # 128 rows per chunk
TC = 16  # time steps per DMA chunk
XSCALE = 4.0  # x shipped as x/XSCALE in fp8, Wi as Wi*XSCALE

# PSUM quad-bank order [g, i, f, o]; quad q -> Wi/Wh column block index
QUAD_COLS = [2, 0, 1, 3]


def build_nc(zero_bias: bool, t_steps: int = T):
    nc = bacc.Bacc("TRN2", target_bir_lowering=False, debug=False, num_devices=NCORES)

    xT = nc.dram_tensor("xT", [3, 128, T, B], dt.float8e4, kind="ExternalInput")
    wiP = nc.dram_tensor("wiP", [4, 2, 128, 2, 128], dt.float8e4, kind="ExternalInput")
    whP = nc.dram_tensor("whP", [4, 128, 2, 128], dt.float8e4, kind="ExternalInput")
    bh4 = nc.dram_tensor("bh4", [128, 4], dt.float32, kind="ExternalInput")
    srcT = nc.dram_tensor("srcT", [128, B], dt.bfloat16, kind="ExternalInput")
    w1b = nc.dram_tensor("w1b", [2, 128, 128], dt.bfloat16, kind="ExternalInput")
    w2b = nc.dram_tensor("w2b", [128, 128], dt.bfloat16, kind="ExternalInput")
    b1 = nc.dram_tensor("b1", [128], dt.float32, kind="ExternalInput")
    b2 = nc.dram_tensor("b2", [128], dt.float32, kind="ExternalInput")
    outT = nc.dram_tensor("outT", [F, B], dt.float32, kind="ExternalOutput")

    nchunk = (t_steps + TC - 1) // TC
    DR = mybir.MatmulPerfMode.DoubleRow

    with tile.TileContext(nc) as tc:
        with (
            tc.tile_pool(name="const", bufs=1) as constp,
            tc.tile_pool(name="gates", bufs=3) as gatep,
        ):
            # ---------------- weights / constants ----------------
            # DMA issue order matters: each dma_start costs ~650ns serialized
            # on the Sync queue, so the tensors gating the first input
            # projection (wi, first x steps) go first; merge-layer constants
            # are only needed ~500us later and go last.
            # junkw/junkx feed PE warm-up matmuls (values irrelevant; Tile
            # requires a write before any read)
            junkw = constp.tile([128, 128], dt.float8e4, name="junkw")
            junkx = constp.tile([128, 512], dt.float8e4, name="junkx")
            nc.gpsimd.memset(junkw[:], 0.0)
            nc.gpsimd.memset(junkx[:], 0.0)

            wi = constp.tile([128, 4, 2, 2, 128], dt.float8e4)
            nc.sync.dma_start(wi[:], wiP[:].rearrange("q pr k two m -> k q pr two m"))

            # ---------------- x staging (double buffer) ----------------
            # No DoubleRow zero-pad plane: the pad half of each DR pass uses
            # a stride-0 AP that re-reads the real data, which is multiplied
            # by the zero weight half (host packs zeros there), so its value
            # never matters and no memset is needed.
            xts = []
            for i in range(2):
                xt = constp.tile([128, 3, TC, B], dt.float8e4, name=f"xt{i}")
                xts.append(xt)

            xsrc = xT[:].rearrange("kc p t b -> p kc t b")
            nc.sync.dma_start(xts[0][:, :, 0:4, :], xsrc[:, :, 0:4, :])
            nc.sync.dma_start(xts[0][:, :, 4:10, :], xsrc[:, :, 4:10, :])

            wh = constp.tile([128, 4, 2, 128], dt.float8e4)
            nc.sync.dma_start(wh[:], whP[:].rearrange("q k two m -> k q two m"))
            nc.sync.dma_start(xts[0][:, :, 10:TC, :], xsrc[:, :, 10:TC, :])

            srcb = constp.tile([128, B], dt.bfloat16)
            nc.sync.dma_start(srcb[:], srcT[:])
            w1 = constp.tile([128, 2, 128], dt.bfloat16)
            nc.sync.dma_start(w1[:], w1b[:].rearrange("two k m -> k two m"))
            w2 = constp.tile([128, 128], dt.bfloat16)
            nc.sync.dma_start(w2[:], w2b[:])
            b1t = constp.tile([128, 1], dt.float32)
            nc.sync.dma_start(b1t[:], b1[:].rearrange("(f one) -> f one", one=1))
            b2t = constp.tile([128, 1], dt.float32)
            nc.sync.dma_start(b2t[:], b2[:].rearrange("(f one) -> f one", one=1))
            bias_g = None
            if not zero_bias:
                bias_g = constp.tile([128, 4], dt.float32)
                nc.sync.dma_start(bias_g[:], bh4[:])

            def dma_chunk(ch):
                t0 = ch * TC
                nc.sync.dma_start(
                    xts[ch % 2][:], xsrc[:, :, t0 : t0 + TC, :]
                )

            def dr_pad(ap):
                """[p, n] AP -> [p, 2, n] with a stride-0 middle dim (the
                DoubleRow second k-tile re-reads the same data; the matching
                stationary half is all zeros)."""
                out = bass.AP(ap.tensor, ap.offset, [ap.ap[0], [0, 2], *ap.ap[1:]])
                return out

            # ---------------- persistent state ----------------
            # hs needs no init: hmul(c, t=0) always writes it before the
            # first rec (t=1) reads it.
            cs, hs = [], []
            for c in range(K):
                c_t = constp.tile([128, NH], dt.bfloat16, name=f"c_{c}")
                nc.gpsimd.memset(c_t[:], 0.0)
                cs.append(c_t)
                hs.append(constp.tile([128, NH], dt.float8e4, name=f"h_{c}"))
            # merge-layer input: the last step's h in bf16, written directly
            # by the final hmul of each chunk
            hbf = constp.tile([128, K, NH], dt.bfloat16)

            zp_ctx = tc.tile_pool(name="zp", bufs=2, space="PSUM")
            zp = zp_ctx.__enter__()

            def emit_ip_pair(zt, t, pr, with_stop):
                """input projection for step t, chunk-pair pr (N=256).

                The pair tile [128, 4, 256] spans two banks (quads g,i and
                f,o); one start per bank (q0/q2), stop on the pair's last
                recurrent matmul per bank (or ip q1/q3 at t=0).
                """
                buf = xts[(t // TC) % 2]
                ts_ = t % TC
                bs = slice(pr * 2 * NH, (pr + 1) * 2 * NH)
                for q in range(4):
                    nc.tensor.matmul(
                        zt[:, q, :],
                        wi[:, q, 0, :, :],
                        buf[:, 0:2, ts_, bs],
                        start=(q in (0, 2)),
                        stop=False,
                        perf_mode=DR,
                        skip_group_check=True,
                    )
                    nc.tensor.matmul(
                        zt[:, q, :],
                        wi[:, q, 1, :, :],
                        dr_pad(buf[:, 2, ts_, bs]),
                        start=False,
                        stop=(with_stop and q in (1, 3)),
                        perf_mode=DR,
                        skip_group_check=True,
                    )

            def emit_rec(zt, c):
                half = slice((c % 2) * NH, (c % 2 + 1) * NH)
                for q in range(4):
                    nc.tensor.matmul(
                        zt[:, q, half],
                        wh[:, q, :, :],
                        dr_pad(hs[c][:]),
                        start=False,
                        stop=(c % 2 == 1 and q in (1, 3)),
                        perf_mode=DR,
                        skip_group_check=True,
                    )

            def emit_gates(zt, t, c):
                """single sigmoid over all 4 quads (g pre-scaled by 2)."""
                half = slice((c % 2) * NH, (c % 2 + 1) * NH)
                sg = gatep.tile(
                    [128, 4, NH], dt.bfloat16, tag=f"sg{c}", name=f"sg{c}_{t}"
                )
                if zero_bias:
                    nc.scalar.activation(sg[:], zt[:, :, half], AF.Sigmoid)
                else:
                    for q in range(4):
                        nc.scalar.activation(
                            sg[:, q, :],
                            zt[:, q, half],
                            AF.Sigmoid,
                            bias=bias_g[:, q : q + 1],
                        )
                return sg

            def emit_cell(t, c, sg):
                # tanh(g) = 2*sig(2g) - 1 (the 2x is pre-folded into the
                # g-quad weights).  Order ts, m1, m2 so no op reads a tile
                # written by the immediately preceding DVE op (the RAW pipe
                # stall costs ~90ns); the final add is deferred one slot for
                # the same reason (see emit_cell_add).
                tg = gatep.tile([128, NH], dt.bfloat16, tag=f"tg{c}", name=f"tg{c}_{t}")
                nc.vector.tensor_scalar(tg[:], sg[:, 0, :], 2.0, -1.0, ALU.mult, ALU.add)
                m1 = gatep.tile([128, NH], dt.bfloat16, tag=f"m1{c}", name=f"m1{c}_{t}")
                nc.vector.tensor_mul(m1[:], sg[:, 2, :], cs[c][:])
                m2 = gatep.tile([128, NH], dt.bfloat16, tag=f"m2{c}", name=f"m2{c}_{t}")
                nc.vector.tensor_mul(m2[:], sg[:, 1, :], tg[:])
                return (m1, m2)

            def emit_cell_add(c, m1, m2):
                nc.vector.tensor_add(cs[c][:], m1[:], m2[:])

            def emit_tail_tanh(t, c, sg):
                tc2 = gatep.tile(
                    [128, NH], dt.bfloat16, tag=f"tc{c}", name=f"tc{c}_{t}"
                )
                nc.scalar.activation(tc2[:], cs[c][:], AF.Tanh)
                return tc2

            def emit_tail_hmul(t, c, sg, tc2):
                # h-mul on the DVE, placed after the slot's cell ops: running
                # it on gpsimd contends with concurrent DVE ops for the
                # shared POOL SBUF port (+200ns on whichever DVE op overlaps).
                # The final step's h goes straight to the merge input in bf16.
                dst = hbf[:, c, :] if t == t_steps - 1 else hs[c][:]
                nc.vector.tensor_mul(dst, sg[:, 3, :], tc2[:])

            # PE warm-up: ~12 junk matmuls on uninitialized scratch get the
            # HAM clock gate to 2.4GHz (~3.4us of PE activity) while the
            # first DMAs land; the first real projection then runs warm.
            # Each is its own closed accumulation group into a region the
            # real ip overwrites with start=True.
            z_cur = []
            for pr in range(K // 2):
                zt = zp.tile([128, 4, 2 * NH], dt.float32, tag=f"zp{pr}", name=f"zp{pr}_p")
                z_cur.append(zt)
            for _ in range(10):
                nc.tensor.matmul(
                    z_cur[0][:, 0:2, :],
                    junkw[:],
                    junkx[:],
                    start=True,
                    stop=True,
                    skip_group_check=True,
                )

            # prologue: projections for t=0
            for pr in range(K // 2):
                emit_ip_pair(z_cur[pr], 0, pr, with_stop=True)

            # Chunk slot order [0,2,1,3]: the rec matmuls running concurrently
            # with sig(c) then always target the OTHER pair's PSUM banks, so
            # the ACT PSUM read never contends with PE bank writes.
            CH = [0, 2, 1, 3]
            # Tails run two slots late and are emitted BEFORE the slot's sig,
            # so the ACT queue always holds ready work (the tanh) while the
            # sig waits on its rec matmuls, and each chunk's rec lands exactly
            # two slots after its h-mul.
            tails = []  # (t, c, sg) awaiting tail emission
            for t in range(t_steps):
                ch = t // TC
                if t % TC == 0 and ch + 1 < nchunk:
                    dma_chunk(ch + 1)
                for ci in range(K):
                    c = CH[ci]
                    hm = None
                    if len(tails) >= 2:
                        hm = tails.pop(0)
                        tc2 = emit_tail_tanh(*hm)
                    zt = z_cur[c // 2]
                    if t > 0:
                        emit_rec(zt, c)
                    # pair ip for t+1 is emitted at the pair's second chunk
                    # slot (both chunks' step t-1 gate reads are then long
                    # retired, so the fresh tile never stalls the PE).
                    if t + 1 < t_steps and c % 2 == 1:
                        pr = c // 2
                        zn = zp.tile(
                            [128, 4, 2 * NH],
                            dt.float32,
                            tag=f"zp{pr}",
                            name=f"zp{pr}_{t + 1}",
                        )
                        emit_ip_pair(zn, t + 1, pr, with_stop=False)
                        z_cur[pr] = zn
                    sg = emit_gates(zt, t, c)
                    m1, m2 = emit_cell(t, c, sg)
                    # hmul between m2 and the add: gives the add pipe
                    # distance from its producers (no DVE RAW stall) while
                    # keeping it in-slot, so the tanh two slots later has
                    # ~2 slots of input margin
                    if hm is not None:
                        emit_tail_hmul(*hm, tc2)
                    emit_cell_add(c, m1, m2)
                    tails.append((t, c, sg))
            for p in tails:
                tc2 = emit_tail_tanh(*p)
                emit_tail_hmul(*p, tc2)

            zp_ctx.__exit__(None, None, None)

            # ---------------- merge layer ----------------
            with tc.tile_pool(name="mp", bufs=1, space="PSUM") as mp:
                ps_hid = mp.tile([128, B], dt.float32)
                for c in range(K):
                    bs = slice(c * NH, (c + 1) * NH)
                    nc.tensor.matmul(
                        ps_hid[:, bs], w1[:, 0, :], hbf[:, c, :], start=True, stop=False
                    )
                    nc.tensor.matmul(
                        ps_hid[:, bs], w1[:, 1, :], srcb[:, bs], start=False, stop=True
                    )
                hid_bf = constp.tile([128, B], dt.bfloat16)
                nc.scalar.activation(hid_bf[:], ps_hid[:], AF.Relu, bias=b1t[:])

                ps_out = mp.tile([128, B], dt.float32)
                nc.tensor.matmul(ps_out[:], w2[:], hid_bf[:], start=True, stop=True)
                out_sb = constp.tile([128, B], dt.float32)
                nc.scalar.activation(out_sb[:], ps_out[:], AF.Identity, bias=b2t[:])
                nc.sync.dma_start(outT[:], out_sb[:])

    nc.compile()
    return nc


_NC_CACHE: dict = {}


def _get_nc(zero_bias: bool):
    if zero_bias not in _NC_CACHE:
        _NC_CACHE[zero_bias] = build_nc(zero_bias)
    return _NC_CACHE[zero_bias]


def make_in_maps(**inputs):
    """Host-side reshaping: slice per core, pre-transpose, pre-quantize."""
    f32 = lambda x: np.asarray(x, dtype=np.float32)
    Wi = f32(inputs["Wi"])  # [384, 512]
    Wh = f32(inputs["Wh"])  # [128, 512]
    bh = f32(inputs["bh"])  # [512]
    W1 = f32(inputs["W1"])  # [256, 128]
    W2 = f32(inputs["W2"])  # [128, 128]
    b1 = f32(inputs["b1"])
    b2 = f32(inputs["b2"])

    # Wi packed for DoubleRow: [q, pair, k, two, m], scaled by XSCALE.
    # Wh packed for DoubleRow with a zero second k-tile: [q, k, two, m].
    # The g quad (and its bias) is additionally scaled by 2 so that
    # tanh(g) = 2*sigmoid(2g) - 1 comes out of the shared sigmoid.
    wiP = np.zeros((4, 2, 128, 2, 128), np.float32)
    whP = np.zeros((4, 128, 2, 128), np.float32)
    bh4 = np.zeros((128, 4), np.float32)
    for q, blk in enumerate(QUAD_COLS):
        gs = 2.0 if q == 0 else 1.0
        colsl = slice(blk * 128, (blk + 1) * 128)
        for kc in range(3):
            wiP[q, kc // 2, :, kc % 2, :] = (
                gs * XSCALE * Wi[kc * 128 : (kc + 1) * 128, colsl]
            )
        whP[q, :, 0, :] = gs * Wh[:, colsl]
        bh4[:, q] = gs * bh[colsl]
    wiP = wiP.astype(F8)
    whP = whP.astype(F8)
    w1b = np.stack([W1[0:128, :], W1[128:256, :]]).astype(BF16)
    w2b = W2.astype(BF16)

    shared = {
        "wiP": wiP,
        "whP": whP,
        "bh4": np.ascontiguousarray(bh4),
        "w1b": w1b,
        "w2b": w2b,
        "b1": b1,
        "b2": b2,
    }

    # big tensors: cast full arrays to fp8 once, then per-core transpose
    planes = []
    for nm in ("seq", "seq_e", "seq_t"):
        a = np.asarray(inputs[nm])
        planes.append((a * (1.0 / XSCALE)).astype(F8))  # [4096, T, F]
    src = f32(inputs["src"])

    in_maps = []
    for c in range(NCORES):
        sl = slice(c * B, (c + 1) * B)
        m = dict(shared)
        xT = np.empty((3, 128, T, B), F8)
        for kc in range(3):
            xT[kc] = planes[kc][sl].transpose(2, 1, 0)
        m["xT"] = xT
        m["srcT"] = np.ascontiguousarray(src[sl].T).astype(BF16)
        in_maps.append(m)
    return in_maps


def kernel(**inputs) -> np.ndarray:
    zero_bias = not np.any(np.asarray(inputs["bh"]))
    nc = _get_nc(zero_bias)
    in_maps = make_in_maps(**inputs)
    res = run_bass_kernel_spmd(nc, in_maps, core_ids=list(range(NCORES)))
    out = np.empty((BFULL, F), np.float32)
    for c in range(NCORES):
        out[c * B : (c + 1) * B] = res.results[c]["outT"].T
    return out



# revision 29
# speedup vs baseline: 1.0037x; 1.0006x over previous
"""LSTM-pool kernel for Trainium2, 8-core data-parallel SPMD.

Math (per batch row b):
  x_t = [seq[b,t], seq_e[b,t], seq_t[b,t]]              (A = 384)
  z_t = x_t @ Wi + h_{t-1} @ Wh + bh                    (4F = 512, gates i,f,g,o)
  c_t = sig(f)*c_{t-1} + sig(i)*tanh(g);  h_t = sig(o)*tanh(c_t)
  out = relu([h_T, src] @ W1 + b1) @ W2 + b2

Implementation notes (steady state is ACT-bound at ~3.69us/step, ACT
99.9% busy = the structural floor for this dataflow):
  * Host pre-transposes the 3 big [B,T,F] tensors to feature-major fp8e4m3
    (x/4, Wi*4 so PSUM holds the exact product) - no on-device casts or
    transposes, 4x less HBM traffic.
  * All z matmuls are fp8 DoubleRow; the pad k-half of each DR pass is a
    stride-0 AP that re-reads real data against an all-zero weight half,
    so no zero-pad planes or memsets exist on device.
  * PSUM: two [128, 4, 256] pair tiles (2 banks each), double buffered =
    all 8 banks.  Exactly one start=True per bank per step.
  * The g-quad columns of Wi/Wh are pre-scaled by 2 on the host so a
    single Sigmoid covers all four gates; tanh(g) = 2*sig(2g)-1 is
    reconstructed on the DVE (one tensor_scalar op).  ACT runs exactly
    4 sigmoids (PSUM-src, FD=512) + 4 tanhs (SBUF-src, FD=128) per step.
  * Batch 512/core is pipelined as K=4 chunks of 128 processed in slot
    order [0,2,1,3], so concurrent rec matmuls never write the PSUM bank
    the sigmoid is reading.  Each chunk's tanh tail runs two slots late
    and is emitted before the slot's sigmoid: the ACT in-order queue then
    always holds ready work while a sigmoid waits on its rec matmuls.
  * The h-mul runs on the DVE between m2 and the cell add - NOT gpsimd,
    whose shared SBUF port would steal ~200ns from whatever DVE op
    overlaps it; its position gives the add pipe distance from its
    producers (no DVE RAW stall).
  * Gate/cell arithmetic is bf16 on the DVE; h is written in fp8 for the
    recurrent matmul, except the last step which writes bf16 straight
    into the merge-layer input (better precision, no copies).
  * Prologue: DMA issue order is latency-critical (each dma_start costs
    ~650ns serialized on the Sync queue): wi first, then the first 4 time
    steps of x, then the rest; merge-layer constants last.  ~10 junk
    matmuls on scratch tiles warm the PE HAM clock gate to 2.4GHz while
    the DMAs land.
"""

import sys

sys.path.insert(0, "/opt/trn_rl_repo")

import numpy as np

import concourse.bass as bass
import concourse.mybir as mybir
import concourse.tile as tile
from concourse import bacc
from concourse.bass_utils import run_bass_kernel_spmd

dt = mybir.dt
AF = mybir.ActivationFunctionType
ALU = mybir.AluOpType
F8 = dt.np(dt.float8e4)
BF16 = dt.np(dt.bfloat16)

NCORES = 8
BFULL = 4096
B = BFULL // NCORES  # 512 batch rows per core
T = 128
F = 128
K = 4  # batch chunks per core
NH = B // K  # 128 rows per chunk
TC = 16  # time steps per DMA chunk
XSCALE = 4.0  # x shipped as x/XSCALE in fp8, Wi as Wi*XSCALE

# PSUM quad-bank order [g, i, f, o]; quad q -> Wi/Wh column block index
QUAD_COLS = [2, 0, 1, 3]


def build_nc(zero_bias: bool, t_steps: int = T):
    nc = bacc.Bacc("TRN2", target_bir_lowering=False, debug=False, num_devices=NCORES)

    xT = nc.dram_tensor("xT", [3, 128, T, B], dt.float8e4, kind="ExternalInput")
    wiP = nc.dram_tensor("wiP", [4, 2, 128, 2, 128], dt.float8e4, kind="ExternalInput")
    whP = nc.dram_tensor("whP", [4, 128, 2, 128], dt.float8e4, kind="ExternalInput")
    bh4 = nc.dram_tensor("bh4", [128, 4], dt.float32, kind="ExternalInput")
    srcT = nc.dram_tensor("srcT", [128, B], dt.bfloat16, kind="ExternalInput")
    w1b = nc.dram_tensor("w1b", [2, 128, 128], dt.bfloat16, kind="ExternalInput")
    w2b = nc.dram_tensor("w2b", [128, 128], dt.bfloat16, kind="ExternalInput")
    b1 = nc.dram_tensor("b1", [128], dt.float32, kind="ExternalInput")
    b2 = nc.dram_tensor("b2", [128], dt.float32, kind="ExternalInput")
    outT = nc.dram_tensor("outT", [F, B], dt.float32, kind="ExternalOutput")

    nchunk = (t_steps + TC - 1) // TC
    DR = mybir.MatmulPerfMode.DoubleRow

    with tile.TileContext(nc) as tc:
        with (
            tc.tile_pool(name="const", bufs=1) as constp,
            tc.tile_pool(name="gates", bufs=3) as gatep,
        ):
            # ---------------- weights / constants ----------------
            # DMA issue order matters: each dma_start costs ~650ns serialized
            # on the Sync queue, so the tensors gating the first input
            # projection (wi, first x steps) go first; merge-layer constants
            # are only needed ~500us later and go last.
            # junkw/junkx feed PE warm-up matmuls (values irrelevant; Tile
            # requires a write before any read)
            junkw = constp.tile([128, 128], dt.float8e4, name="junkw")
            junkx = constp.tile([128, 512], dt.float8e4, name="junkx")
            nc.gpsimd.memset(junkw[:], 0.0)
            nc.gpsimd.memset(junkx[:], 0.0)

            wi = constp.tile([128, 4, 2, 2, 128], dt.float8e4)
            nc.sync.dma_start(wi[:], wiP[:].rearrange("q pr k two m -> k q pr two m"))

            # ---------------- x staging (double buffer) ----------------
            # No DoubleRow zero-pad plane: the pad half of each DR pass uses
            # a stride-0 AP that re-reads the real data, which is multiplied
            # by the zero weight half (host packs zeros there), so its value
            # never matters and no memset is needed.
            xts = []
            for i in range(2):
                xt = constp.tile([128, 3, TC, B], dt.float8e4, name=f"xt{i}")
                xts.append(xt)

            xsrc = xT[:].rearrange("kc p t b -> p kc t b")
            nc.sync.dma_start(xts[0][:, :, 0:4, :], xsrc[:, :, 0:4, :])
            nc.sync.dma_start(xts[0][:, :, 4:10, :], xsrc[:, :, 4:10, :])

            wh = constp.tile([128, 4, 2, 128], dt.float8e4)
            nc.sync.dma_start(wh[:], whP[:].rearrange("q k two m -> k q two m"))
            nc.sync.dma_start(xts[0][:, :, 10:TC, :], xsrc[:, :, 10:TC, :])

            srcb = constp.tile([128, B], dt.bfloat16)
            nc.sync.dma_start(srcb[:], srcT[:])
            w1 = constp.tile([128, 2, 128], dt.bfloat16)
            nc.sync.dma_start(w1[:], w1b[:].rearrange("two k m -> k two m"))
            w2 = constp.tile([128, 128], dt.bfloat16)
            nc.sync.dma_start(w2[:], w2b[:])
            b1t = constp.tile([128, 1], dt.float32)
            nc.sync.dma_start(b1t[:], b1[:].rearrange("(f one) -> f one", one=1))
            b2t = constp.tile([128, 1], dt.float32)
            nc.sync.dma_start(b2t[:], b2[:].rearrange("(f one) -> f one", one=1))
            bias_g = None
            if not zero_bias:
                bias_g = constp.tile([128, 4], dt.float32)
                nc.sync.dma_start(bias_g[:], bh4[:])

            def dma_chunk(ch):
                t0 = ch * TC
                nc.sync.dma_start(
                    xts[ch % 2][:], xsrc[:, :, t0 : t0 + TC, :]
                )

            def dr_pad(ap):
                """[p, n] AP -> [p, 2, n] with a stride-0 middle dim (the
                DoubleRow second k-tile re-reads the same data; the matching
                stationary half is all zeros)."""
                out = bass.AP(ap.tensor, ap.offset, [ap.ap[0], [0, 2], *ap.ap[1:]])
                return out

            # ---------------- persistent state ----------------
            # hs needs no init: hmul(c, t=0) always writes it before the
            # first rec (t=1) reads it.
            cs, hs = [], []
            for c in range(K):
                c_t = constp.tile([128, NH], dt.bfloat16, name=f"c_{c}")
                nc.gpsimd.memset(c_t[:], 0.0)
                cs.append(c_t)
                hs.append(constp.tile([128, NH], dt.float8e4, name=f"h_{c}"))
            # merge-layer input: the last step's h in bf16, written directly
            # by the final hmul of each chunk
            hbf = constp.tile([128, K, NH], dt.bfloat16)

            zp_ctx = tc.tile_pool(name="zp", bufs=2, space="PSUM")
            zp = zp_ctx.__enter__()

            def emit_ip_pair(zt, t, pr, with_stop):
                """input projection for step t, chunk-pair pr (N=256).

                The pair tile [128, 4, 256] spans two banks (quads g,i and
                f,o); one start per bank (q0/q2), stop on the pair's last
                recurrent matmul per bank (or ip q1/q3 at t=0).
                """
                buf = xts[(t // TC) % 2]
                ts_ = t % TC
                bs = slice(pr * 2 * NH, (pr + 1) * 2 * NH)
                for q in range(4):
                    nc.tensor.matmul(
                        zt[:, q, :],
                        wi[:, q, 0, :, :],
                        buf[:, 0:2, ts_, bs],
                        start=(q in (0, 2)),
                        stop=False,
                        perf_mode=DR,
                        skip_group_check=True,
                    )
                    nc.tensor.matmul(
                        zt[:, q, :],
                        wi[:, q, 1, :, :],
                        dr_pad(buf[:, 2, ts_, bs]),
                        start=False,
                        stop=(with_stop and q in (1, 3)),
                        perf_mode=DR,
                        skip_group_check=True,
                    )

            def emit_rec(zt, c):
                half = slice((c % 2) * NH, (c % 2 + 1) * NH)
                for q in range(4):
                    nc.tensor.matmul(
                        zt[:, q, half],
                        wh[:, q, :, :],
                        dr_pad(hs[c][:]),
                        start=False,
                        stop=(c % 2 == 1 and q in (1, 3)),
                        perf_mode=DR,
                        skip_group_check=True,
                    )

            def emit_gates(zt, t, c):
                """single sigmoid over all 4 quads (g pre-scaled by 2)."""
                half = slice((c % 2) * NH, (c % 2 + 1) * NH)
                sg = gatep.tile(
                    [128, 4, NH], dt.bfloat16, tag=f"sg{c}", name=f"sg{c}_{t}"
                )
                if zero_bias:
                    nc.scalar.activation(sg[:], zt[:, :, half], AF.Sigmoid)
                else:
                    for q in range(4):
                        nc.scalar.activation(
                            sg[:, q, :],
                            zt[:, q, half],
                            AF.Sigmoid,
                            bias=bias_g[:, q : q + 1],
                        )
                return sg

            def emit_cell(t, c, sg):
                # tanh(g) = 2*sig(2g) - 1 (the 2x is pre-folded into the
                # g-quad weights).  Order ts, m1, m2 so no op reads a tile
                # written by the immediately preceding DVE op (the RAW pipe
                # stall costs ~90ns); the final add is deferred one slot for
                # the same reason (see emit_cell_add).
                tg = gatep.tile([128, NH], dt.bfloat16, tag=f"tg{c}", name=f"tg{c}_{t}")
                nc.vector.tensor_scalar(tg[:], sg[:, 0, :], 2.0, -1.0, ALU.mult, ALU.add)
                m1 = gatep.tile([128, NH], dt.bfloat16, tag=f"m1{c}", name=f"m1{c}_{t}")
                nc.vector.tensor_mul(m1[:], sg[:, 2, :], cs[c][:])
                m2 = gatep.tile([128, NH], dt.bfloat16, tag=f"m2{c}", name=f"m2{c}_{t}")
                nc.vector.tensor_mul(m2[:], sg[:, 1, :], tg[:])
                return (m1, m2)

            def emit_cell_add(c, m1, m2):
                nc.vector.tensor_add(cs[c][:], m1[:], m2[:])

            def emit_tail_tanh(t, c, sg):
                # fp32 dest: probing whether the tanh's +52ns over model is
                # the ACT write-side bubble on a small bf16 SBUF destination
                tc2 = gatep.tile(
                    [128, NH], dt.float32, tag=f"tc{c}", name=f"tc{c}_{t}"
                )
                nc.scalar.activation(tc2[:], cs[c][:], AF.Tanh)
                return tc2

            def emit_tail_hmul(t, c, sg, tc2):
                # h-mul on the DVE, placed after the slot's cell ops: running
                # it on gpsimd contends with concurrent DVE ops for the
                # shared POOL SBUF port (+200ns on whichever DVE op overlaps).
                # The final step's h goes straight to the merge input in bf16.
                dst = hbf[:, c, :] if t == t_steps - 1 else hs[c][:]
                nc.vector.tensor_mul(dst, sg[:, 3, :], tc2[:])

            # PE warm-up: ~12 junk matmuls on uninitialized scratch get the
            # HAM clock gate to 2.4GHz (~3.4us of PE activity) while the
            # first DMAs land; the first real projection then runs warm.
            # Each is its own closed accumulation group into a region the
            # real ip overwrites with start=True.
            z_cur = []
            for pr in range(K // 2):
                zt = zp.tile([128, 4, 2 * NH], dt.float32, tag=f"zp{pr}", name=f"zp{pr}_p")
                z_cur.append(zt)
            for _ in range(10):
                nc.tensor.matmul(
                    z_cur[0][:, 0:2, :],
                    junkw[:],
                    junkx[:],
                    start=True,
                    stop=True,
                    skip_group_check=True,
                )

            # prologue: projections for t=0
            for pr in range(K // 2):
                emit_ip_pair(z_cur[pr], 0, pr, with_stop=True)

            # Chunk slot order [0,2,1,3]: the rec matmuls running concurrently
            # with sig(c) then always target the OTHER pair's PSUM banks, so
            # the ACT PSUM read never contends with PE bank writes.
            CH = [0, 2, 1, 3]
            # Tails run two slots late and are emitted BEFORE the slot's sig,
            # so the ACT queue always holds ready work (the tanh) while the
            # sig waits on its rec matmuls, and each chunk's rec lands exactly
            # two slots after its h-mul.
            tails = []  # (t, c, sg) awaiting tail emission
            for t in range(t_steps):
                ch = t // TC
                if t % TC == 0 and ch + 1 < nchunk:
                    dma_chunk(ch + 1)
                for ci in range(K):
                    c = CH[ci]
                    hm = None
                    if len(tails) >= 2:
                        hm = tails.pop(0)
                        tc2 = emit_tail_tanh(*hm)
                    zt = z_cur[c // 2]
                    if t > 0:
                        emit_rec(zt, c)
                    # pair ip for t+1 is emitted at the pair's second chunk
                    # slot (both chunks' step t-1 gate reads are then long
                    # retired, so the fresh tile never stalls the PE).
                    if t + 1 < t_steps and c % 2 == 1:
                        pr = c // 2
                        zn = zp.tile(
                            [128, 4, 2 * NH],
                            dt.float32,
                            tag=f"zp{pr}",
                            name=f"zp{pr}_{t + 1}",
                        )
                        emit_ip_pair(zn, t + 1, pr, with_stop=False)
                        z_cur[pr] = zn
                    sg = emit_gates(zt, t, c)
                    m1, m2 = emit_cell(t, c, sg)
                    # hmul between m2 and the add: gives the add pipe
                    # distance from its producers (no DVE RAW stall) while
                    # keeping it in-slot, so the tanh two slots later has
                    # ~2 slots of input margin
                    if hm is not None:
                        emit_tail_hmul(*hm, tc2)
                    emit_cell_add(c, m1, m2)
                    tails.append((t, c, sg))
            for p in tails:
                tc2 = emit_tail_tanh(*p)
                emit_tail_hmul(*p, tc2)

            zp_ctx.__exit__(None, None, None)

            # ---------------- merge layer ----------------
            with tc.tile_pool(name="mp", bufs=1, space="PSUM") as mp:
                ps_hid = mp.tile([128, B], dt.float32)
                for c in range(K):
                    bs = slice(c * NH, (c + 1) * NH)
                    nc.tensor.matmul(
                        ps_hid[:, bs], w1[:, 0, :], hbf[:, c, :], start=True, stop=False
                    )
                    nc.tensor.matmul(
                        ps_hid[:, bs], w1[:, 1, :], srcb[:, bs], start=False, stop=True
                    )
                hid_bf = constp.tile([128, B], dt.bfloat16)
                nc.scalar.activation(hid_bf[:], ps_hid[:], AF.Relu, bias=b1t[:])

                ps_out = mp.tile([128, B], dt.float32)
                nc.tensor.matmul(ps_out[:], w2[:], hid_bf[:], start=True, stop=True)
                out_sb = constp.tile([128, B], dt.float32)
                nc.scalar.activation(out_sb[:], ps_out[:], AF.Identity, bias=b2t[:])
                nc.sync.dma_start(outT[:], out_sb[:])

    nc.compile()
    return nc


_NC_CACHE: dict = {}


def _get_nc(zero_bias: bool):
    if zero_bias not in _NC_CACHE:
        _NC_CACHE[zero_bias] = build_nc(zero_bias)
    return _NC_CACHE[zero_bias]


def make_in_maps(**inputs):
    """Host-side reshaping: slice per core, pre-transpose, pre-quantize."""
    f32 = lambda x: np.asarray(x, dtype=np.float32)
    Wi = f32(inputs["Wi"])  # [384, 512]
    Wh = f32(inputs["Wh"])  # [128, 512]
    bh = f32(inputs["bh"])  # [512]
    W1 = f32(inputs["W1"])  # [256, 128]
    W2 = f32(inputs["W2"])  # [128, 128]
    b1 = f32(inputs["b1"])
    b2 = f32(inputs["b2"])

    # Wi packed for DoubleRow: [q, pair, k, two, m], scaled by XSCALE.
    # Wh packed for DoubleRow with a zero second k-tile: [q, k, two, m].
    # The g quad (and its bias) is additionally scaled by 2 so that
    # tanh(g) = 2*sigmoid(2g) - 1 comes out of the shared sigmoid.
    wiP = np.zeros((4, 2, 128, 2, 128), np.float32)
    whP = np.zeros((4, 128, 2, 128), np.float32)
    bh4 = np.zeros((128, 4), np.float32)
    for q, blk in enumerate(QUAD_COLS):
        gs = 2.0 if q == 0 else 1.0
        colsl = slice(blk * 128, (blk + 1) * 128)
        for kc in range(3):
            wiP[q, kc // 2, :, kc % 2, :] = (
                gs * XSCALE * Wi[kc * 128 : (kc + 1) * 128, colsl]
            )
        whP[q, :, 0, :] = gs * Wh[:, colsl]
        bh4[:, q] = gs * bh[colsl]
    wiP = wiP.astype(F8)
    whP = whP.astype(F8)
    w1b = np.stack([W1[0:128, :], W1[128:256, :]]).astype(BF16)
    w2b = W2.astype(BF16)

    shared = {
        "wiP": wiP,
        "whP": whP,
        "bh4": np.ascontiguousarray(bh4),
        "w1b": w1b,
        "w2b": w2b,
        "b1": b1,
        "b2": b2,
    }

    # big tensors: cast full arrays to fp8 once, then per-core transpose
    planes = []
    for nm in ("seq", "seq_e", "seq_t"):
        a = np.asarray(inputs[nm])
        planes.append((a * (1.0 / XSCALE)).astype(F8))  # [4096, T, F]
    src = f32(inputs["src"])

    in_maps = []
    for c in range(NCORES):
        sl = slice(c * B, (c + 1) * B)
        m = dict(shared)
        xT = np.empty((3, 128, T, B), F8)
        for kc in range(3):
            xT[kc] = planes[kc][sl].transpose(2, 1, 0)
        m["xT"] = xT
        m["srcT"] = np.ascontiguousarray(src[sl].T).astype(BF16)
        in_maps.append(m)
    return in_maps


def kernel(**inputs) -> np.ndarray:
    zero_bias = not np.any(np.asarray(inputs["bh"]))
    nc = _get_nc(zero_bias)
    in_maps = make_in_maps(**inputs)
    res = run_bass_kernel_spmd(nc, in_maps, core_ids=list(range(NCORES)))
    out = np.empty((BFULL, F), np.float32)
    for c in range(NCORES):
        out[c * B : (c + 1) * B] = res.results[c]["outT"].T
    return out

